# revision 2
# baseline (speedup 1.0000x reference)
"""AdaFocalLoss on 8 Trainium2 NeuronCores.

Strategy (data-parallel, per the sharding hint):
  - shard the 65536 logit rows across 8 cores (8192 rows each)
  - per core, stream 2 MB chunks of logits; one ScalarE pass computes
    exp(x) with accum_out (per-row sum of exps); one VectorE
    scalar_tensor_tensor pass computes sum_c((iota==target)*x) = the
    target-class logit, also via accum_out. Both ride the same DMA.
  - tail (per-row, [128, 64]): lse=ln(sumexp), logpt=x_t-lse,
    pt=exp(logpt), gamma sign/mag looked up via a telescoped
    sum_b(delta_b * [pt >= b/15]) chain, loss=-(1-s*pt+eps)^m * logpt,
    reduced to one scalar per core with a PE matmul against ones.
  - host sums the 8 per-core partial scalars (the gather/unshard step).
"""

import sys

for _p in ("/opt/trn_rl_repo",):
    if _p not in sys.path:
        sys.path.insert(0, _p)

import numpy as np

NUM_BINS = 15
EPS = 1e-20
N, C = 65536, 1000
NCORES = 8
NSHARD = N // NCORES  # 8192 rows per core
P = 128
KROWS = 4  # rows per partition per DMA chunk
CHUNK = P * KROWS  # 512 rows = 2 MB per chunk
T = NSHARD // CHUNK  # 16 chunks
R = NSHARD // P  # 64 row-slots per partition


def _split_excess_waits(nc, mybir, max_waits=1):
    """This container's walrus supports only one sync-wait command per
    instruction; hoist extra waits onto preceding same-engine no-ops."""
    ctr = 0
    for f in nc.m.functions:
        for bb in f.blocks:
            new_insts = []
            changed = False
            for inst in bb.instructions:
                si = inst.sync_info
                if si is not None and si.on_wait and len(si.on_wait) > max_waits:
                    waits = list(si.on_wait)
                    excess, keep = waits[:-max_waits], waits[-max_waits:]
                    for i in range(0, len(excess), max_waits):
                        ctr += 1
                        new_insts.append(
                            mybir.InstNoOp(
                                name=f"I-waitsplit-{ctr}",
                                sync_info=mybir.SyncInfo(
                                    on_wait=list(excess[i : i + max_waits]),
                                    on_update=[],
                                ),
                                bass_nofuse=True,
                                engine=inst.engine,
                            )
                        )
                    si.on_wait = keep
                    changed = True
                new_insts.append(inst)
            if changed:
                bb.instructions[:] = new_insts


def _build():
    import concourse.bass as bass
    import concourse.tile as tile
    from concourse import mybir

    f32 = mybir.dt.float32
    AF = mybir.ActivationFunctionType
    ALU = mybir.AluOpType
    NB = NUM_BINS

    nc = bass.Bass()
    x = nc.declare_dram_parameter("x", [NSHARD, C], f32, isOutput=False)
    tmap = nc.declare_dram_parameter("tmap", [P, R], f32, isOutput=False)
    iota = nc.declare_dram_parameter("iota", [P, C], f32, isOutput=False)
    gb = nc.declare_dram_parameter("gb", [P, NB], f32, isOutput=False)
    out = nc.declare_dram_parameter("out", [1, 1], f32, isOutput=True)

    x3 = x[:].rearrange("(t p k) c -> t p (k c)", t=T, p=P, k=KROWS)

    with tile.TileContext(nc) as tc:
        with (
            tc.tile_pool(name="const", bufs=1) as cpool,
            tc.tile_pool(name="io", bufs=4) as iopool,
            tc.tile_pool(name="escr", bufs=2) as epool,
            tc.tile_pool(name="sscr", bufs=2) as spool,
            tc.tile_pool(name="acc", bufs=1) as apool,
            tc.tile_pool(name="tail", bufs=3) as tpool,
            tc.tile_pool(name="psum", bufs=1, space="PSUM") as ppool,
        ):
            iota_t = cpool.tile([P, C], f32, tag="iota")
            nc.sync.dma_start(iota_t[:], iota[:])
            tmap_t = cpool.tile([P, R], f32, tag="tmap")
            nc.sync.dma_start(tmap_t[:], tmap[:])
            gb_t = cpool.tile([P, NB], f32, tag="gb")
            nc.sync.dma_start(gb_t[:], gb[:])

            sumexp = apool.tile([P, R], f32, tag="sumexp")
            xt = apool.tile([P, R], f32, tag="xt")

            for t in range(T):
                xtile = iopool.tile([P, KROWS * C], f32, tag="xtile")
                nc.sync.dma_start(xtile[:], x3[t, :, :])
                for k in range(KROWS):
                    slot = t * KROWS + k
                    sub = xtile[:, k * C : (k + 1) * C]
                    eo = epool.tile([P, C], f32, tag="eo")
                    nc.scalar.activation(
                        eo[:], sub, AF.Exp, accum_out=sumexp[:, slot : slot + 1]
                    )
                    so = spool.tile([P, C], f32, tag="so")
                    nc.vector.scalar_tensor_tensor(
                        so[:],
                        iota_t[:],
                        tmap_t[:, slot : slot + 1],
                        sub,
                        ALU.is_equal,
                        ALU.mult,
                        accum_out=xt[:, slot : slot + 1],
                    )

            # ---- per-row tail on [P, R] ----
            lse = tpool.tile([P, R], f32, tag="lse")
            nc.scalar.activation(lse[:], sumexp[:], AF.Ln)
            logpt = tpool.tile([P, R], f32, tag="logpt")
            nc.vector.tensor_sub(logpt[:], xt[:], lse[:])
            pt = tpool.tile([P, R], f32, tag="pt")
            nc.scalar.activation(pt[:], logpt[:], AF.Exp)

            # gamma sign/magnitude tables and their telescoped deltas
            sgn = tpool.tile([P, NB], f32, tag="sgn")
            nc.scalar.activation(sgn[:], gb_t[:], AF.Sign)
            mag = tpool.tile([P, NB], f32, tag="mag")
            nc.scalar.activation(mag[:], gb_t[:], AF.Abs)
            ds = tpool.tile([P, NB], f32, tag="ds")
            nc.vector.tensor_copy(ds[:, 0:1], sgn[:, 0:1])
            nc.vector.tensor_sub(ds[:, 1:NB], sgn[:, 1:NB], sgn[:, 0 : NB - 1])
            dm = tpool.tile([P, NB], f32, tag="dm")
            nc.vector.tensor_copy(dm[:, 0:1], mag[:, 0:1])
            nc.vector.tensor_sub(dm[:, 1:NB], mag[:, 1:NB], mag[:, 0 : NB - 1])

            # s(pt) = sum_b ds_b * [pt >= b/15]; m(pt) likewise
            s_acc = tpool.tile([P, R], f32, tag="s_acc")
            nc.vector.memset(s_acc[:], 0.0)
            m_acc = tpool.tile([P, R], f32, tag="m_acc")
            nc.vector.memset(m_acc[:], 0.0)
            for b in range(NB):
                mask = tpool.tile([P, R], f32, tag="mask")
                nc.vector.tensor_scalar(
                    mask[:], pt[:], float(b) / NB, None, ALU.is_ge
                )
                s_new = tpool.tile([P, R], f32, tag="s_acc")
                nc.vector.scalar_tensor_tensor(
                    s_new[:], mask[:], ds[:, b : b + 1], s_acc[:], ALU.mult, ALU.add
                )
                m_new = tpool.tile([P, R], f32, tag="m_acc")
                nc.vector.scalar_tensor_tensor(
                    m_new[:], mask[:], dm[:, b : b + 1], m_acc[:], ALU.mult, ALU.add
                )
                s_acc, m_acc = s_new, m_new

            # u = 1 + eps - s*pt ;  y = u^m = exp(m * ln(u))
            nspt = tpool.tile([P, R], f32, tag="nspt")
            nc.vector.scalar_tensor_tensor(
                nspt[:], s_acc[:], -1.0, pt[:], ALU.mult, ALU.mult
            )
            u = tpool.tile([P, R], f32, tag="u")
            nc.vector.tensor_scalar(u[:], nspt[:], 1.0 + EPS, None, ALU.add)
            v = tpool.tile([P, R], f32, tag="v")
            nc.scalar.activation(v[:], u[:], AF.Ln)
            w = tpool.tile([P, R], f32, tag="w")
            nc.vector.tensor_mul(w[:], v[:], m_acc[:])
            y = tpool.tile([P, R], f32, tag="y")
            nc.scalar.activation(y[:], w[:], AF.Exp)

            # rowsum[p] = sum_j y*logpt (negated on host)
            prod = tpool.tile([P, R], f32, tag="prod")
            nc.vector.tensor_mul(prod[:], y[:], logpt[:])
            rowsum = tpool.tile([P, 1], f32, tag="rowsum")
            nc.vector.tensor_reduce(
                rowsum[:], prod[:], mybir.AxisListType.X, ALU.add
            )

            ones = tpool.tile([P, 1], f32, tag="ones")
            nc.vector.memset(ones[:], 1.0)
            ps = ppool.tile([1, 1], f32, tag="ps")
            nc.tensor.matmul(ps[:], ones[:], rowsum[:], start=True, stop=True)
            res = tpool.tile([1, 1], f32, tag="res")
            nc.scalar.copy(res[:], ps[:])
            nc.sync.dma_start(out[:], res[:])

    _split_excess_waits(nc, mybir, max_waits=1)
    return nc


_NC = None


def _get_nc():
    global _NC
    if _NC is None:
        _NC = _build()
    return _NC


def _make_in_maps(input, target, gammas):
    inp = np.ascontiguousarray(np.asarray(input, dtype=np.float32))
    tgt = np.asarray(target).astype(np.int64)
    gam = np.asarray(gammas, dtype=np.float32)
    assert inp.shape == (N, C) and tgt.shape == (N,) and gam.shape == (NUM_BINS,)

    iota_const = np.ascontiguousarray(
        np.broadcast_to(np.arange(C, dtype=np.float32), (P, C))
    )
    gb_const = np.ascontiguousarray(np.broadcast_to(gam, (P, NUM_BINS)))

    in_maps = []
    for i in range(NCORES):
        shard = inp[NSHARD * i : NSHARD * (i + 1)]
        tshard = tgt[NSHARD * i : NSHARD * (i + 1)]
        tmap = np.ascontiguousarray(
            tshard.reshape(T, P, KROWS).transpose(1, 0, 2).reshape(P, R)
        ).astype(np.float32)
        in_maps.append(
            {"x": shard, "tmap": tmap, "iota": iota_const, "gb": gb_const}
        )
    return in_maps


def kernel(input, target, gammas, _trace=False, _tmpdir=None):
    from concourse.bass_utils import run_bass_kernel_spmd

    nc = _get_nc()
    in_maps = _make_in_maps(input, target, gammas)
    res = run_bass_kernel_spmd(
        nc,
        in_maps,
        core_ids=list(range(NCORES)),
        trace=_trace,
        tmpdir=_tmpdir,
    )
    partials = [float(res.results[i]["out"][0, 0]) for i in range(NCORES)]
    total = -np.float32(np.sum(np.asarray(partials, dtype=np.float32)))
    if _trace:
        kernel._last_result = res
    return np.array(total, dtype=np.float32)


# revision 7
# speedup vs baseline: 1.1325x; 1.1325x over previous
"""AdaFocalLoss on 8 Trainium2 NeuronCores.

Strategy (data-parallel, per the sharding hint):
  - shard the 65536 logit rows across 8 cores (8192 rows each)
  - per core, stream 2 MB chunks of logits; one ScalarE pass computes
    exp(x) with accum_out (per-row sum of exps); one VectorE
    scalar_tensor_tensor pass computes sum_c((iota==target)*x) = the
    target-class logit, also via accum_out. Both ride the same DMA.
  - tail (per-row, [128, 64]): lse=ln(sumexp), logpt=x_t-lse,
    pt=exp(logpt), gamma sign/mag looked up via a telescoped
    sum_b(delta_b * [pt >= b/15]) chain, loss=-(1-s*pt+eps)^m * logpt,
    reduced to one scalar per core with a PE matmul against ones.
  - host sums the 8 per-core partial scalars (the gather/unshard step).
"""

import sys

for _p in ("/opt/trn_rl_repo",):
    if _p not in sys.path:
        sys.path.insert(0, _p)

import numpy as np

NUM_BINS = 15
EPS = 1e-20
N, C = 65536, 1000
NCORES = 8
NSHARD = N // NCORES  # 8192 rows per core
P = 128
KROWS = 4  # rows per partition per DMA chunk
CHUNK = P * KROWS  # 512 rows = 2 MB per chunk
T = NSHARD // CHUNK  # 16 chunks
R = NSHARD // P  # 64 row-slots per partition


def _split_excess_waits(nc, mybir, max_waits=1):
    """This container's walrus supports only one sync-wait command per
    instruction; hoist extra waits onto preceding same-engine no-ops."""
    ctr = 0
    for f in nc.m.functions:
        for bb in f.blocks:
            new_insts = []
            changed = False
            for inst in bb.instructions:
                si = inst.sync_info
                if si is not None and si.on_wait and len(si.on_wait) > max_waits:
                    waits = list(si.on_wait)
                    excess, keep = waits[:-max_waits], waits[-max_waits:]
                    for i in range(0, len(excess), max_waits):
                        ctr += 1
                        new_insts.append(
                            mybir.InstNoOp(
                                name=f"I-waitsplit-{ctr}",
                                sync_info=mybir.SyncInfo(
                                    on_wait=list(excess[i : i + max_waits]),
                                    on_update=[],
                                ),
                                bass_nofuse=True,
                                engine=inst.engine,
                            )
                        )
                    si.on_wait = keep
                    changed = True
                new_insts.append(inst)
            if changed:
                bb.instructions[:] = new_insts


def _build():
    import concourse.bass as bass
    import concourse.tile as tile
    from concourse import mybir

    f32 = mybir.dt.float32
    f16 = mybir.dt.float16
    AF = mybir.ActivationFunctionType
    ALU = mybir.AluOpType
    NB = NUM_BINS

    nc = bass.Bass()
    x = nc.declare_dram_parameter("x", [NSHARD, C], f32, isOutput=False)
    tmap = nc.declare_dram_parameter("tmap", [P, R], f32, isOutput=False)
    iota = nc.declare_dram_parameter("iota", [P, C], f16, isOutput=False)
    gb = nc.declare_dram_parameter("gb", [P, NB], f32, isOutput=False)
    out = nc.declare_dram_parameter("out", [1, 1], f32, isOutput=True)

    x3 = x[:].rearrange("(t p k) c -> t p (k c)", t=T, p=P, k=KROWS)

    with tile.TileContext(nc) as tc:
        with (
            tc.tile_pool(name="const", bufs=1) as cpool,
            tc.tile_pool(name="io", bufs=6) as iopool,
            tc.tile_pool(name="escr", bufs=3) as epool,
            tc.tile_pool(name="sscr", bufs=3) as spool,
            tc.tile_pool(name="acc", bufs=1) as apool,
            tc.tile_pool(name="tail", bufs=3) as tpool,
            tc.tile_pool(name="psum", bufs=1, space="PSUM") as ppool,
        ):
            iota_t = cpool.tile([P, C], f16, tag="iota")
            nc.sync.dma_start(iota_t[:], iota[:])
            tmap_t = cpool.tile([P, R], f32, tag="tmap")
            nc.sync.dma_start(tmap_t[:], tmap[:])
            gb_t = cpool.tile([P, NB], f32, tag="gb")
            nc.sync.dma_start(gb_t[:], gb[:])

            sumexp = apool.tile([P, R], f32, tag="sumexp")
            xtexp = apool.tile([P, R], f32, tag="xtexp")

            for t in range(T):
                xtile = iopool.tile([P, KROWS * C], f32, tag="xtile")
                nc.sync.dma_start(xtile[:], x3[t, :, :])
                for k in range(KROWS):
                    slot = t * KROWS + k
                    sub = xtile[:, k * C : (k + 1) * C]
                    # exp in fp16 so the gather below runs in DVE 2x mode;
                    # the row-sum of exps accumulates in f32 on the side
                    eo = epool.tile([P, C], f16, tag="eo")
                    nc.scalar.activation(
                        eo[:], sub, AF.Exp, accum_out=sumexp[:, slot : slot + 1]
                    )
                    # xtexp[p] = sum_c (iota==target_p) * exp(x)[p,c]
                    so = spool.tile([P, C], f16, tag="so")
                    nc.vector.scalar_tensor_tensor(
                        so[:],
                        iota_t[:],
                        tmap_t[:, slot : slot + 1],
                        eo[:],
                        ALU.is_equal,
                        ALU.mult,
                        accum_out=xtexp[:, slot : slot + 1],
                    )

            # ---- per-row tail on [P, R] ----
            lse = tpool.tile([P, R], f32, tag="lse")
            nc.scalar.activation(lse[:], sumexp[:], AF.Ln)
            ln_xt = tpool.tile([P, R], f32, tag="ln_xt")
            nc.scalar.activation(ln_xt[:], xtexp[:], AF.Ln)
            logpt = tpool.tile([P, R], f32, tag="logpt")
            nc.vector.tensor_sub(logpt[:], ln_xt[:], lse[:])
            pt = tpool.tile([P, R], f32, tag="pt")
            nc.scalar.activation(pt[:], logpt[:], AF.Exp)

            # gamma sign/magnitude tables and their telescoped deltas
            sgn = tpool.tile([P, NB], f32, tag="sgn")
            nc.scalar.activation(sgn[:], gb_t[:], AF.Sign)
            mag = tpool.tile([P, NB], f32, tag="mag")
            nc.scalar.activation(mag[:], gb_t[:], AF.Abs)
            ds = tpool.tile([P, NB], f32, tag="ds")
            nc.vector.tensor_copy(ds[:, 0:1], sgn[:, 0:1])
            nc.vector.tensor_sub(ds[:, 1:NB], sgn[:, 1:NB], sgn[:, 0 : NB - 1])
            dm = tpool.tile([P, NB], f32, tag="dm")
            nc.vector.tensor_copy(dm[:, 0:1], mag[:, 0:1])
            nc.vector.tensor_sub(dm[:, 1:NB], mag[:, 1:NB], mag[:, 0 : NB - 1])

            # s(pt) = sum_b ds_b * [pt >= b/15]; m(pt) likewise
            s_acc = tpool.tile([P, R], f32, tag="s_acc")
            nc.vector.memset(s_acc[:], 0.0)
            m_acc = tpool.tile([P, R], f32, tag="m_acc")
            nc.vector.memset(m_acc[:], 0.0)
            for b in range(NB):
                mask = tpool.tile([P, R], f32, tag="mask")
                nc.vector.tensor_scalar(
                    mask[:], pt[:], float(b) / NB, None, ALU.is_ge
                )
                s_new = tpool.tile([P, R], f32, tag="s_acc")
                nc.vector.scalar_tensor_tensor(
                    s_new[:], mask[:], ds[:, b : b + 1], s_acc[:], ALU.mult, ALU.add
                )
                m_new = tpool.tile([P, R], f32, tag="m_acc")
                nc.vector.scalar_tensor_tensor(
                    m_new[:], mask[:], dm[:, b : b + 1], m_acc[:], ALU.mult, ALU.add
                )
                s_acc, m_acc = s_new, m_new

            # u = 1 + eps - s*pt ;  y = u^m = exp(m * ln(u))
            nspt = tpool.tile([P, R], f32, tag="nspt")
            nc.vector.scalar_tensor_tensor(
                nspt[:], s_acc[:], -1.0, pt[:], ALU.mult, ALU.mult
            )
            u = tpool.tile([P, R], f32, tag="u")
            nc.vector.tensor_scalar(u[:], nspt[:], 1.0 + EPS, None, ALU.add)
            v = tpool.tile([P, R], f32, tag="v")
            nc.scalar.activation(v[:], u[:], AF.Ln)
            w = tpool.tile([P, R], f32, tag="w")
            nc.vector.tensor_mul(w[:], v[:], m_acc[:])
            y = tpool.tile([P, R], f32, tag="y")
            nc.scalar.activation(y[:], w[:], AF.Exp)

            # rowsum[p] = sum_j y*logpt (negated on host)
            prod = tpool.tile([P, R], f32, tag="prod")
            nc.vector.tensor_mul(prod[:], y[:], logpt[:])
            rowsum = tpool.tile([P, 1], f32, tag="rowsum")
            nc.vector.tensor_reduce(
                rowsum[:], prod[:], mybir.AxisListType.X, ALU.add
            )

            ones = tpool.tile([P, 1], f32, tag="ones")
            nc.vector.memset(ones[:], 1.0)
            ps = ppool.tile([1, 1], f32, tag="ps")
            nc.tensor.matmul(ps[:], ones[:], rowsum[:], start=True, stop=True)
            res = tpool.tile([1, 1], f32, tag="res")
            nc.scalar.copy(res[:], ps[:])
            nc.sync.dma_start(out[:], res[:])

    _split_excess_waits(nc, mybir, max_waits=1)
    return nc


_NC = None


def _get_nc():
    global _NC
    if _NC is None:
        _NC = _build()
    return _NC


def _make_in_maps(input, target, gammas):
    inp = np.ascontiguousarray(np.asarray(input, dtype=np.float32))
    tgt = np.asarray(target).astype(np.int64)
    gam = np.asarray(gammas, dtype=np.float32)
    assert inp.shape == (N, C) and tgt.shape == (N,) and gam.shape == (NUM_BINS,)

    iota_const = np.ascontiguousarray(
        np.broadcast_to(np.arange(C, dtype=np.float16), (P, C))
    )
    gb_const = np.ascontiguousarray(np.broadcast_to(gam, (P, NUM_BINS)))

    in_maps = []
    for i in range(NCORES):
        shard = inp[NSHARD * i : NSHARD * (i + 1)]
        tshard = tgt[NSHARD * i : NSHARD * (i + 1)]
        tmap = np.ascontiguousarray(
            tshard.reshape(T, P, KROWS).transpose(1, 0, 2).reshape(P, R)
        ).astype(np.float32)
        in_maps.append(
            {"x": shard, "tmap": tmap, "iota": iota_const, "gb": gb_const}
        )
    return in_maps


def kernel(input, target, gammas, _trace=False, _tmpdir=None):
    from concourse.bass_utils import run_bass_kernel_spmd

    nc = _get_nc()
    in_maps = _make_in_maps(input, target, gammas)
    res = run_bass_kernel_spmd(
        nc,
        in_maps,
        core_ids=list(range(NCORES)),
        trace=_trace,
        tmpdir=_tmpdir,
    )
    partials = [float(res.results[i]["out"][0, 0]) for i in range(NCORES)]
    total = -np.float32(np.sum(np.asarray(partials, dtype=np.float32)))
    if _trace:
        kernel._last_result = res
    return np.array(total, dtype=np.float32)
